# revision 10
# baseline (speedup 1.0000x reference)
"""Trainium2 Bass kernel for nn_DotPredictor (GNN edge MLP).

Computes, for E edges over node features h [N, 128]:
    x = h[src] * h[dst]                  # per-edge gather + elementwise mul
    x = relu(x @ W1 + b1)                # 128 -> 256
    x = relu(x @ W2 + b2)                # 256 -> 256
    out = x @ W3 + b3                    # 256 -> 1

Strategy (8 NeuronCores, SPMD single program):
  - Edges sharded across the 8 cores; h (cast to bf16) + weights replicated.
  - Per-edge rows of h are gathered with gpsimd.dma_gather(transpose=True),
    which lands features on partitions ([feat, edge] layout) so no on-chip
    transpose is ever needed.  dma_gather indices are int16, so h is split
    into 4 row-chunks of 32768 and edges are bucketed host-side by
    (src_chunk, dst_chunk); outputs are un-permuted on the host.
  - MLP runs in bf16 with fp32 PSUM accumulation, hidden-on-partition
    layout: y1T[h,e] = W1_chunk.T @ VT, etc.  Bias+ReLU are fused into the
    PSUM->SBUF eviction (ScalarE activation / VectorE tensor_scalar).
"""

import os
import sys
from contextlib import ExitStack

import numpy as np
import ml_dtypes

for _p in ("/opt/trn_rl_repo", "/root/.axon_site/_ro/trn_rl_repo"):
    if os.path.isdir(_p) and _p not in sys.path:
        sys.path.insert(0, _p)

import concourse.bacc as bacc
import concourse.bass as bass
import concourse.tile as tile
from concourse import mybir
from concourse.masks import make_identity
from concourse.bass_utils import run_bass_kernel_spmd

BF16 = mybir.dt.bfloat16
F32 = mybir.dt.float32
I16 = mybir.dt.int16

N_NODES = 100000
N_EDGES = 1600000
F = 128          # in_feats
H = 256          # hidden
NCORES = 8
CHUNK = 32768    # h row-chunk so local indices fit int16
TILE = 512       # edges per matmul tile
GBMAX = 8192     # max indices per dma_gather instruction

_cache = {}


def _plan(src, dst, n_nodes, ncores, chunk, tile):
    """Bucket edges by (src_chunk, dst_chunk); equal padded per-core layout.

    Returns dict with order, per-core wrapped idx arrays, caps, counts, etc.
    """
    nchunks = -(-n_nodes // chunk)
    nb = nchunks * nchunks
    src = src.astype(np.int64)
    dst = dst.astype(np.int64)
    bucket = (src // chunk) * nchunks + (dst // chunk)
    order = np.argsort(bucket, kind="stable")
    sizes = np.bincount(bucket, minlength=nb)
    starts = np.concatenate([[0], np.cumsum(sizes)[:-1]])
    # per-core split of each bucket
    counts = sizes[None, :] // ncores + (
        np.arange(ncores)[:, None] < (sizes[None, :] % ncores)
    )  # [ncores, nb]
    offs = np.cumsum(counts, axis=0) - counts
    caps = -(-counts.max(axis=0) // tile) * tile  # per-core bucket capacity
    caps = np.maximum(caps, tile)
    cap_starts = np.concatenate([[0], np.cumsum(caps)[:-1]])
    n_core = int(caps.sum())

    src_s = src[order]
    dst_s = dst[order]
    isrc_w = np.zeros((ncores, 128, n_core // 16), np.int16)
    idst_w = np.zeros((ncores, 128, n_core // 16), np.int16)
    for c in range(ncores):
        loc_s = np.zeros(n_core, np.int16)
        loc_d = np.zeros(n_core, np.int16)
        for b in range(nb):
            n = int(counts[c, b])
            if n == 0:
                continue
            s0 = int(starts[b] + offs[c, b])
            p0 = int(cap_starts[b])
            loc_s[p0 : p0 + n] = src_s[s0 : s0 + n] - (b // nchunks) * chunk
            loc_d[p0 : p0 + n] = dst_s[s0 : s0 + n] - (b % nchunks) * chunk
        # wrap: idx k -> partition k%16, column k//16; replicate to 8 groups
        ws = loc_s.reshape(-1, 16).T
        wd = loc_d.reshape(-1, 16).T
        isrc_w[c] = np.tile(ws, (8, 1))
        idst_w[c] = np.tile(wd, (8, 1))

    return dict(
        nchunks=nchunks, nb=nb, order=order, starts=starts, counts=counts,
        offs=offs, caps=caps, cap_starts=cap_starts, n_core=n_core,
        isrc_w=isrc_w, idst_w=idst_w,
    )


def _build(n_nodes, chunk, caps, n_core, b3_val, gbmax=GBMAX, tilesz=TILE):
    """Trace + compile the SPMD Tile program (same for all cores)."""
    nchunks = -(-n_nodes // chunk)
    nb = len(caps)
    under_axon = True  # no /dev/neuron* on the client pod
    nc = bacc.Bacc("TRN2", target_bir_lowering=False, debug=False,
                   num_devices=NCORES, num_swdge_queues=4)

    h_t = nc.dram_tensor("hb", [n_nodes, F], BF16, kind="ExternalInput")
    w1_t = nc.dram_tensor("w1", [F, H], BF16, kind="ExternalInput")
    w2_t = nc.dram_tensor("w2", [H, H], BF16, kind="ExternalInput")
    w3_t = nc.dram_tensor("w3", [128, 2], BF16, kind="ExternalInput")
    b1_t = nc.dram_tensor("b1", [128, 2], F32, kind="ExternalInput")
    b2_t = nc.dram_tensor("b2", [128, 2], F32, kind="ExternalInput")
    b3_t = nc.dram_tensor("b3", [1, 1], F32, kind="ExternalInput")
    is_t = nc.dram_tensor("isrc", [128, n_core // 16], I16, kind="ExternalInput")
    id_t = nc.dram_tensor("idst", [128, n_core // 16], I16, kind="ExternalInput")
    out_t = nc.dram_tensor("out", [1, n_core], F32, kind="ExternalOutput")

    # gather blocks: (bucket, pos, n) — pos/n multiples of tilesz
    blocks = []
    for b in range(nb):
        pos = int(np.concatenate([[0], np.cumsum(caps)[:-1]])[b])
        left = int(caps[b])
        off = 0
        while left > 0:
            n = min(gbmax, left)
            blocks.append((b, pos + off, n))
            off += n
            left -= n

    with ExitStack() as ctx:
        tc = ctx.enter_context(tile.TileContext(nc))
        wpool = ctx.enter_context(tc.tile_pool(name="weights", bufs=1))
        ipool = ctx.enter_context(tc.tile_pool(name="idx", bufs=8))
        gpool = ctx.enter_context(tc.tile_pool(name="gather", bufs=4))
        apool = ctx.enter_context(tc.tile_pool(name="acts", bufs=3))
        spool = ctx.enter_context(tc.tile_pool(name="stage", bufs=4))
        p1pool = ctx.enter_context(tc.tile_pool(name="py1", bufs=2, space="PSUM"))
        p2pool = ctx.enter_context(tc.tile_pool(name="py2", bufs=1, space="PSUM"))
        popool = ctx.enter_context(tc.tile_pool(name="pout", bufs=1, space="PSUM"))
        ptpool = ctx.enter_context(tc.tile_pool(name="pvt", bufs=1, space="PSUM"))

        w1s = wpool.tile([F, H], BF16)
        nc.sync.dma_start(w1s[:], w1_t.ap())
        w2s = []
        for k in range(2):
            t = wpool.tile([128, H], BF16, tag=f"w2_{k}")
            nc.sync.dma_start(t[:], w2_t.ap()[k * 128 : (k + 1) * 128, :])
            w2s.append(t)
        w3s = wpool.tile([128, 2], BF16)
        nc.sync.dma_start(w3s[:], w3_t.ap())
        b1s = wpool.tile([128, 2], F32)
        nc.sync.dma_start(b1s[:], b1_t.ap())
        b2s = wpool.tile([128, 2], F32)
        nc.sync.dma_start(b2s[:], b2_t.ap())
        b3s = wpool.tile([1, 1], F32)
        nc.sync.dma_start(b3s[:], b3_t.ap())

        ident = wpool.tile([128, 128], BF16, tag="ident")
        make_identity(nc, ident[:])

        tl = 0  # global tile counter (for engine ping-pong)
        gq = 0  # SWDGE queue rotation (4 queues = 4 Q7 core pairs)
        for (b, pos, n) in blocks:
            ci, cj = b // nchunks, b % nchunks
            r0s, r0d = ci * chunk, cj * chunk
            rss = min(chunk, n_nodes - r0s)
            rsd = min(chunk, n_nodes - r0d)

            ist_t = ipool.tile([128, n // 16], I16, tag="isrc")
            nc.sync.dma_start(ist_t[:], is_t.ap()[:, pos // 16 : (pos + n) // 16])
            idt_t = ipool.tile([128, n // 16], I16, tag="idst")
            nc.sync.dma_start(idt_t[:], id_t.ap()[:, pos // 16 : (pos + n) // 16])
            ist = ist_t[:]
            idt = idt_t[:]

            # non-transpose gathers: edge k -> partition k%128, column k//128.
            # These run concurrently across SWDGE queues (distinct Q7 pairs).
            S = gpool.tile([128, n // 128, F], BF16, tag="S")
            D = gpool.tile([128, n // 128, F], BF16, tag="D")
            nc.gpsimd.dma_gather(
                S[:], h_t.ap()[r0s : r0s + rss, :], ist,
                num_idxs=n, num_idxs_reg=n, elem_size=F, transpose=False,
                single_packet=False, queue_num=gq % 4,
            )
            nc.gpsimd.dma_gather(
                D[:], h_t.ap()[r0d : r0d + rsd, :], idt,
                num_idxs=n, num_idxs_reg=n, elem_size=F, transpose=False,
                single_packet=False, queue_num=(gq + 1) % 4,
            )
            gq += 2
            # V = S * D in place (bf16, 2x mode), [edge, feat] layout
            nc.vector.tensor_tensor(
                out=S[:].rearrange("p a b -> p (a b)"),
                in0=S[:].rearrange("p a b -> p (a b)"),
                in1=D[:].rearrange("p a b -> p (a b)"),
                op=mybir.AluOpType.mult,
            )

            for t in range(n // tilesz):
                e0 = t * tilesz
                # PE-transpose the 4 [128e,128f] blocks of this tile into
                # feat-on-partition layout; edge order is preserved.
                vt_ps = ptpool.tile([128, tilesz], BF16, tag="vt")
                for c in range(tilesz // 128):
                    nc.tensor.transpose(
                        out=vt_ps[:, c * 128 : (c + 1) * 128],
                        in_=S[:, t * (tilesz // 128) + c, :],
                        identity=ident[:],
                    )
                vt_sb = apool.tile([128, tilesz], BF16, tag="vt_sb")
                nc.vector.tensor_copy(out=vt_sb[:], in_=vt_ps[:])
                vt = vt_sb[:]
                y1 = p1pool.tile([128, 2, tilesz], F32, tag="y1")
                for m in range(2):
                    nc.tensor.matmul(
                        y1[:, m, :], lhsT=w1s[:, m * 128 : (m + 1) * 128],
                        rhs=vt, start=True, stop=True,
                    )
                x2 = apool.tile([128, 2, tilesz], BF16, tag="x2")
                # fused bias+relu eviction, split across ACT / DVE
                am = tl % 2  # which chunk ACT takes (ping-pong for balance)
                nc.scalar.activation(
                    x2[:, am, :], y1[:, am, :],
                    mybir.ActivationFunctionType.Relu, bias=b1s[:, am : am + 1],
                )
                nc.vector.tensor_scalar(
                    out=x2[:, 1 - am, :], in0=y1[:, 1 - am, :],
                    scalar1=b1s[:, 1 - am : 2 - am], scalar2=0.0,
                    op0=mybir.AluOpType.add, op1=mybir.AluOpType.max,
                )
                y2 = p2pool.tile([128, 2, tilesz], F32, tag="y2")
                for m in range(2):
                    for k in range(2):
                        nc.tensor.matmul(
                            y2[:, m, :], lhsT=w2s[k][:, m * 128 : (m + 1) * 128],
                            rhs=x2[:, k, :], start=(k == 0), stop=(k == 1),
                        )
                x3 = apool.tile([128, 2, tilesz], BF16, tag="x3")
                nc.scalar.activation(
                    x3[:, 1 - am, :], y2[:, 1 - am, :],
                    mybir.ActivationFunctionType.Relu,
                    bias=b2s[:, 1 - am : 2 - am],
                )
                nc.vector.tensor_scalar(
                    out=x3[:, am, :], in0=y2[:, am, :],
                    scalar1=b2s[:, am : am + 1], scalar2=0.0,
                    op0=mybir.AluOpType.add, op1=mybir.AluOpType.max,
                )
                op = popool.tile([1, tilesz], F32, tag="po")
                for k in range(2):
                    nc.tensor.matmul(
                        op[:1, :], lhsT=w3s[:, k : k + 1], rhs=x3[:, k, :],
                        start=(k == 0), stop=(k == 1),
                    )
                st = spool.tile([1, tilesz], F32, tag="st")
                nc.scalar.activation(
                    st[:1, :], op[:1, :],
                    mybir.ActivationFunctionType.Identity, bias=b3s[:1, :1],
                )
                nc.scalar.dma_start(out_t.ap()[:, pos + e0 : pos + e0 + tilesz],
                                    st[:1, :])
                tl += 1

    nc.compile()
    return nc


def _prep_shared(h, W1, W2, W3, b1, b2):
    bf = ml_dtypes.bfloat16
    hb = np.ascontiguousarray(h.astype(bf))
    w1 = np.ascontiguousarray(W1.astype(bf))
    w2 = np.ascontiguousarray(W2.astype(bf))
    w3 = np.ascontiguousarray(W3.reshape(2, 128).T.astype(bf))
    b1p = np.ascontiguousarray(b1.reshape(2, 128).T.astype(np.float32))
    b2p = np.ascontiguousarray(b2.reshape(2, 128).T.astype(np.float32))
    return hb, w1, w2, w3, b1p, b2p


def run(h, src, dst, W1, b1, W2, b2, W3, b3, *,
        n_nodes=None, ncores=NCORES, chunk=CHUNK, tilesz=TILE, gbmax=GBMAX,
        trace=False):
    """Full pipeline. Returns (out [E,1] float32, BassKernelResults)."""
    n_nodes = n_nodes or h.shape[0]
    E = src.shape[0]
    plan = _plan(np.asarray(src), np.asarray(dst), n_nodes, ncores, chunk, tilesz)
    hb, w1, w2, w3, b1p, b2p = _prep_shared(
        np.asarray(h), np.asarray(W1), np.asarray(W2), np.asarray(W3),
        np.asarray(b1), np.asarray(b2))

    key = (n_nodes, chunk, tuple(plan["caps"]), plan["n_core"], float(b3[0]),
           gbmax, tilesz)
    if key not in _cache:
        _cache[key] = _build(n_nodes, chunk, plan["caps"], plan["n_core"],
                             float(b3[0]), gbmax, tilesz)
    nc = _cache[key]

    in_maps = []
    for c in range(ncores):
        in_maps.append({
            "hb": hb, "w1": w1, "w2": w2, "w3": w3, "b1": b1p, "b2": b2p,
            "b3": np.asarray(b3, np.float32).reshape(1, 1),
            "isrc": plan["isrc_w"][c], "idst": plan["idst_w"][c],
        })
    res = run_bass_kernel_spmd(nc, in_maps, list(range(ncores)), trace=trace)

    # reassemble
    nb = plan["nb"]
    counts, offs = plan["counts"], plan["offs"]
    starts, cap_starts = plan["starts"], plan["cap_starts"]
    out_sorted = np.empty(E, np.float32)
    for c in range(ncores):
        o = res.results[c]["out"].reshape(-1)
        for b in range(nb):
            n = int(counts[c, b])
            if n == 0:
                continue
            s0 = int(starts[b] + offs[c, b])
            p0 = int(cap_starts[b])
            out_sorted[s0 : s0 + n] = o[p0 : p0 + n]
    out = np.empty(E, np.float32)
    out[plan["order"]] = out_sorted
    return out.reshape(E, 1), res


def kernel(**inputs) -> np.ndarray:
    out, _ = run(**{k: np.asarray(v) for k, v in inputs.items()})
    return out


# revision 11
# speedup vs baseline: 1.1007x; 1.1007x over previous
"""Trainium2 Bass kernel for nn_DotPredictor (GNN edge MLP).

Computes, for E edges over node features h [N, 128]:
    x = h[src] * h[dst]                  # per-edge gather + elementwise mul
    x = relu(x @ W1 + b1)                # 128 -> 256
    x = relu(x @ W2 + b2)                # 256 -> 256
    out = x @ W3 + b3                    # 256 -> 1

Strategy (8 NeuronCores, SPMD single program):
  - Edges sharded across the 8 cores; h (cast to bf16) + weights replicated.
  - Per-edge rows of h are gathered with gpsimd.dma_gather(transpose=True),
    which lands features on partitions ([feat, edge] layout) so no on-chip
    transpose is ever needed.  dma_gather indices are int16, so h is split
    into 4 row-chunks of 32768 and edges are bucketed host-side by
    (src_chunk, dst_chunk); outputs are un-permuted on the host.
  - MLP runs in bf16 with fp32 PSUM accumulation, hidden-on-partition
    layout: y1T[h,e] = W1_chunk.T @ VT, etc.  Bias+ReLU are fused into the
    PSUM->SBUF eviction (ScalarE activation / VectorE tensor_scalar).
"""

import os
import sys
from contextlib import ExitStack

import numpy as np
import ml_dtypes

for _p in ("/opt/trn_rl_repo", "/root/.axon_site/_ro/trn_rl_repo"):
    if os.path.isdir(_p) and _p not in sys.path:
        sys.path.insert(0, _p)

import concourse.bacc as bacc
import concourse.bass as bass
import concourse.tile as tile
from concourse import mybir
from concourse.masks import make_identity
from concourse.bass_utils import run_bass_kernel_spmd

BF16 = mybir.dt.bfloat16
F32 = mybir.dt.float32
I16 = mybir.dt.int16

N_NODES = 100000
N_EDGES = 1600000
F = 128          # in_feats
H = 256          # hidden
NCORES = 8
CHUNK = 32768    # h row-chunk so local indices fit int16
TILE = 512       # edges per matmul tile
GBMAX = 4096     # max indices per dma_gather instruction

_cache = {}


def _plan(src, dst, n_nodes, ncores, chunk, tile):
    """Bucket edges by (src_chunk, dst_chunk); equal padded per-core layout.

    Returns dict with order, per-core wrapped idx arrays, caps, counts, etc.
    """
    nchunks = -(-n_nodes // chunk)
    nb = nchunks * nchunks
    src = src.astype(np.int64)
    dst = dst.astype(np.int64)
    bucket = (src // chunk) * nchunks + (dst // chunk)
    order = np.argsort(bucket, kind="stable")
    sizes = np.bincount(bucket, minlength=nb)
    starts = np.concatenate([[0], np.cumsum(sizes)[:-1]])
    # per-core split of each bucket
    counts = sizes[None, :] // ncores + (
        np.arange(ncores)[:, None] < (sizes[None, :] % ncores)
    )  # [ncores, nb]
    offs = np.cumsum(counts, axis=0) - counts
    caps = -(-counts.max(axis=0) // tile) * tile  # per-core bucket capacity
    caps = np.maximum(caps, tile)
    cap_starts = np.concatenate([[0], np.cumsum(caps)[:-1]])
    n_core = int(caps.sum())

    src_s = src[order]
    dst_s = dst[order]
    isrc_w = np.zeros((ncores, 128, n_core // 16), np.int16)
    idst_w = np.zeros((ncores, 128, n_core // 16), np.int16)
    for c in range(ncores):
        loc_s = np.zeros(n_core, np.int16)
        loc_d = np.zeros(n_core, np.int16)
        for b in range(nb):
            n = int(counts[c, b])
            if n == 0:
                continue
            s0 = int(starts[b] + offs[c, b])
            p0 = int(cap_starts[b])
            loc_s[p0 : p0 + n] = src_s[s0 : s0 + n] - (b // nchunks) * chunk
            loc_d[p0 : p0 + n] = dst_s[s0 : s0 + n] - (b % nchunks) * chunk
        # wrap: idx k -> partition k%16, column k//16; replicate to 8 groups
        ws = loc_s.reshape(-1, 16).T
        wd = loc_d.reshape(-1, 16).T
        isrc_w[c] = np.tile(ws, (8, 1))
        idst_w[c] = np.tile(wd, (8, 1))

    return dict(
        nchunks=nchunks, nb=nb, order=order, starts=starts, counts=counts,
        offs=offs, caps=caps, cap_starts=cap_starts, n_core=n_core,
        isrc_w=isrc_w, idst_w=idst_w,
    )


def _build(n_nodes, chunk, caps, n_core, b3_val, gbmax=GBMAX, tilesz=TILE):
    """Trace + compile the SPMD Tile program (same for all cores)."""
    nchunks = -(-n_nodes // chunk)
    nb = len(caps)
    under_axon = True  # no /dev/neuron* on the client pod
    nc = bacc.Bacc("TRN2", target_bir_lowering=False, debug=False,
                   num_devices=NCORES, num_swdge_queues=4)

    h_t = nc.dram_tensor("hb", [n_nodes, F], BF16, kind="ExternalInput")
    w1_t = nc.dram_tensor("w1", [F, H], BF16, kind="ExternalInput")
    w2_t = nc.dram_tensor("w2", [H, H], BF16, kind="ExternalInput")
    w3_t = nc.dram_tensor("w3", [128, 2], BF16, kind="ExternalInput")
    b1_t = nc.dram_tensor("b1", [128, 2], F32, kind="ExternalInput")
    b2_t = nc.dram_tensor("b2", [128, 2], F32, kind="ExternalInput")
    b3_t = nc.dram_tensor("b3", [1, 1], F32, kind="ExternalInput")
    is_t = nc.dram_tensor("isrc", [128, n_core // 16], I16, kind="ExternalInput")
    id_t = nc.dram_tensor("idst", [128, n_core // 16], I16, kind="ExternalInput")
    out_t = nc.dram_tensor("out", [1, n_core], F32, kind="ExternalOutput")

    # gather blocks: (bucket, pos, n) — pos/n multiples of tilesz
    blocks = []
    for b in range(nb):
        pos = int(np.concatenate([[0], np.cumsum(caps)[:-1]])[b])
        left = int(caps[b])
        off = 0
        while left > 0:
            n = min(gbmax, left)
            blocks.append((b, pos + off, n))
            off += n
            left -= n

    with ExitStack() as ctx:
        tc = ctx.enter_context(tile.TileContext(nc))
        wpool = ctx.enter_context(tc.tile_pool(name="weights", bufs=1))
        gpool = ctx.enter_context(tc.tile_pool(name="gather", bufs=7))
        apool = ctx.enter_context(tc.tile_pool(name="acts", bufs=3))
        spool = ctx.enter_context(tc.tile_pool(name="stage", bufs=4))
        p1pool = ctx.enter_context(tc.tile_pool(name="py1", bufs=2, space="PSUM"))
        p2pool = ctx.enter_context(tc.tile_pool(name="py2", bufs=1, space="PSUM"))
        popool = ctx.enter_context(tc.tile_pool(name="pout", bufs=1, space="PSUM"))
        ptpool = ctx.enter_context(tc.tile_pool(name="pvt", bufs=1, space="PSUM"))

        w1s = wpool.tile([F, H], BF16)
        nc.sync.dma_start(w1s[:], w1_t.ap())
        w2s = []
        for k in range(2):
            t = wpool.tile([128, H], BF16, tag=f"w2_{k}")
            nc.sync.dma_start(t[:], w2_t.ap()[k * 128 : (k + 1) * 128, :])
            w2s.append(t)
        w3s = wpool.tile([128, 2], BF16)
        nc.sync.dma_start(w3s[:], w3_t.ap())
        b1s = wpool.tile([128, 2], F32)
        nc.sync.dma_start(b1s[:], b1_t.ap())
        b2s = wpool.tile([128, 2], F32)
        nc.sync.dma_start(b2s[:], b2_t.ap())
        b3s = wpool.tile([1, 1], F32)
        nc.sync.dma_start(b3s[:], b3_t.ap())

        ident = wpool.tile([128, 128], BF16, tag="ident")
        make_identity(nc, ident[:])

        # whole idx arrays resident in SBUF: removes all per-block idx deps
        ist_all = wpool.tile([128, n_core // 16], I16, tag="ist_all")
        nc.sync.dma_start(ist_all[:], is_t.ap())
        idt_all = wpool.tile([128, n_core // 16], I16, tag="idt_all")
        nc.sync.dma_start(idt_all[:], id_t.ap())

        tl = 0  # global tile counter (for engine ping-pong)
        gq = 0  # SWDGE queue rotation (4 queues = 4 Q7 core pairs)
        for (b, pos, n) in blocks:
            ci, cj = b // nchunks, b % nchunks
            r0s, r0d = ci * chunk, cj * chunk
            rss = min(chunk, n_nodes - r0s)
            rsd = min(chunk, n_nodes - r0d)

            ist = ist_all[:, pos // 16 : (pos + n) // 16]
            idt = idt_all[:, pos // 16 : (pos + n) // 16]

            # non-transpose gathers: edge k -> partition k%128, column k//128.
            # These run concurrently across SWDGE queues (distinct Q7 pairs).
            S = gpool.tile([128, n // 128, F], BF16, tag="S")
            D = gpool.tile([128, n // 128, F], BF16, tag="D")
            nc.gpsimd.dma_gather(
                S[:], h_t.ap()[r0s : r0s + rss, :], ist,
                num_idxs=n, num_idxs_reg=n, elem_size=F, transpose=False,
                single_packet=False, queue_num=gq % 4,
            )
            nc.gpsimd.dma_gather(
                D[:], h_t.ap()[r0d : r0d + rsd, :], idt,
                num_idxs=n, num_idxs_reg=n, elem_size=F, transpose=False,
                single_packet=False, queue_num=(gq + 1) % 4,
            )
            gq += 2
            # V = S * D in place (bf16, 2x mode), [edge, feat] layout
            nc.vector.tensor_tensor(
                out=S[:].rearrange("p a b -> p (a b)"),
                in0=S[:].rearrange("p a b -> p (a b)"),
                in1=D[:].rearrange("p a b -> p (a b)"),
                op=mybir.AluOpType.mult,
            )

            for t in range(n // tilesz):
                e0 = t * tilesz
                # PE-transpose the 4 [128e,128f] blocks of this tile into
                # feat-on-partition layout; edge order is preserved.
                vt_ps = ptpool.tile([128, tilesz], BF16, tag="vt")
                for c in range(tilesz // 128):
                    nc.tensor.transpose(
                        out=vt_ps[:, c * 128 : (c + 1) * 128],
                        in_=S[:, t * (tilesz // 128) + c, :],
                        identity=ident[:],
                    )
                vt_sb = apool.tile([128, tilesz], BF16, tag="vt_sb")
                nc.vector.tensor_copy(out=vt_sb[:], in_=vt_ps[:])
                vt = vt_sb[:]
                y1 = p1pool.tile([128, 2, tilesz], F32, tag="y1")
                for m in range(2):
                    nc.tensor.matmul(
                        y1[:, m, :], lhsT=w1s[:, m * 128 : (m + 1) * 128],
                        rhs=vt, start=True, stop=True,
                    )
                x2 = apool.tile([128, 2, tilesz], BF16, tag="x2")
                # fused bias+relu eviction, split across ACT / DVE
                am = tl % 2  # which chunk ACT takes (ping-pong for balance)
                nc.scalar.activation(
                    x2[:, am, :], y1[:, am, :],
                    mybir.ActivationFunctionType.Relu, bias=b1s[:, am : am + 1],
                )
                nc.vector.tensor_scalar(
                    out=x2[:, 1 - am, :], in0=y1[:, 1 - am, :],
                    scalar1=b1s[:, 1 - am : 2 - am], scalar2=0.0,
                    op0=mybir.AluOpType.add, op1=mybir.AluOpType.max,
                )
                y2 = p2pool.tile([128, 2, tilesz], F32, tag="y2")
                for m in range(2):
                    for k in range(2):
                        nc.tensor.matmul(
                            y2[:, m, :], lhsT=w2s[k][:, m * 128 : (m + 1) * 128],
                            rhs=x2[:, k, :], start=(k == 0), stop=(k == 1),
                        )
                x3 = apool.tile([128, 2, tilesz], BF16, tag="x3")
                nc.scalar.activation(
                    x3[:, 1 - am, :], y2[:, 1 - am, :],
                    mybir.ActivationFunctionType.Relu,
                    bias=b2s[:, 1 - am : 2 - am],
                )
                nc.vector.tensor_scalar(
                    out=x3[:, am, :], in0=y2[:, am, :],
                    scalar1=b2s[:, am : am + 1], scalar2=0.0,
                    op0=mybir.AluOpType.add, op1=mybir.AluOpType.max,
                )
                op = popool.tile([1, tilesz], F32, tag="po")
                for k in range(2):
                    nc.tensor.matmul(
                        op[:1, :], lhsT=w3s[:, k : k + 1], rhs=x3[:, k, :],
                        start=(k == 0), stop=(k == 1),
                    )
                st = spool.tile([1, tilesz], F32, tag="st")
                nc.scalar.activation(
                    st[:1, :], op[:1, :],
                    mybir.ActivationFunctionType.Identity, bias=b3s[:1, :1],
                )
                nc.scalar.dma_start(out_t.ap()[:, pos + e0 : pos + e0 + tilesz],
                                    st[:1, :])
                tl += 1

    nc.compile()
    return nc


def _prep_shared(h, W1, W2, W3, b1, b2):
    bf = ml_dtypes.bfloat16
    hb = np.ascontiguousarray(h.astype(bf))
    w1 = np.ascontiguousarray(W1.astype(bf))
    w2 = np.ascontiguousarray(W2.astype(bf))
    w3 = np.ascontiguousarray(W3.reshape(2, 128).T.astype(bf))
    b1p = np.ascontiguousarray(b1.reshape(2, 128).T.astype(np.float32))
    b2p = np.ascontiguousarray(b2.reshape(2, 128).T.astype(np.float32))
    return hb, w1, w2, w3, b1p, b2p


def run(h, src, dst, W1, b1, W2, b2, W3, b3, *,
        n_nodes=None, ncores=NCORES, chunk=CHUNK, tilesz=TILE, gbmax=GBMAX,
        trace=False):
    """Full pipeline. Returns (out [E,1] float32, BassKernelResults)."""
    n_nodes = n_nodes or h.shape[0]
    E = src.shape[0]
    plan = _plan(np.asarray(src), np.asarray(dst), n_nodes, ncores, chunk, tilesz)
    hb, w1, w2, w3, b1p, b2p = _prep_shared(
        np.asarray(h), np.asarray(W1), np.asarray(W2), np.asarray(W3),
        np.asarray(b1), np.asarray(b2))

    key = (n_nodes, chunk, tuple(plan["caps"]), plan["n_core"], float(b3[0]),
           gbmax, tilesz)
    if key not in _cache:
        _cache[key] = _build(n_nodes, chunk, plan["caps"], plan["n_core"],
                             float(b3[0]), gbmax, tilesz)
    nc = _cache[key]

    in_maps = []
    for c in range(ncores):
        in_maps.append({
            "hb": hb, "w1": w1, "w2": w2, "w3": w3, "b1": b1p, "b2": b2p,
            "b3": np.asarray(b3, np.float32).reshape(1, 1),
            "isrc": plan["isrc_w"][c], "idst": plan["idst_w"][c],
        })
    res = run_bass_kernel_spmd(nc, in_maps, list(range(ncores)), trace=trace)

    # reassemble
    nb = plan["nb"]
    counts, offs = plan["counts"], plan["offs"]
    starts, cap_starts = plan["starts"], plan["cap_starts"]
    out_sorted = np.empty(E, np.float32)
    for c in range(ncores):
        o = res.results[c]["out"].reshape(-1)
        for b in range(nb):
            n = int(counts[c, b])
            if n == 0:
                continue
            s0 = int(starts[b] + offs[c, b])
            p0 = int(cap_starts[b])
            out_sorted[s0 : s0 + n] = o[p0 : p0 + n]
    out = np.empty(E, np.float32)
    out[plan["order"]] = out_sorted
    return out.reshape(E, 1), res


def kernel(**inputs) -> np.ndarray:
    out, _ = run(**{k: np.asarray(v) for k, v in inputs.items()})
    return out
